# revision 17
# baseline (speedup 1.0000x reference)
"""Trainium2 Bass kernel for nn_ConvShiftLayer.

Computes, per batch element n:
    h = x[n] @ W_dense + b_dense                      (2048, 2048)
    y[t, o] = sum_{d=0..7} h[t-d, (o+d) % 2048]       (h[<0] = 0)
    a = tanh(y),  z = (y > 0) as f32
Returns (y, a, z) each of shape (8, 2048, 2048) f32.

Strategy: data-parallel over batch, 1 element per NeuronCore (8 cores).
Per core:
  - x row-blocks are PE-transposed to xT (D on partitions), one block per
    tile iteration, written into that iteration's chunk PSUM tiles before
    the dense matmuls claim them - so TensorE never waits on the full x DMA.
  - h = xT.T @ W via fp32r matmuls (full-rate) into PSUM; kept there.
  - The 8-tap shifted sum uses the factorization
        y = (1 + S + S^2 + S^3)(1 + S^4) h,   S q[t,o] = q[t-1, o+1]
    as 4 shift-matrix matmuls per 512-chunk (q = h + S^4 h, then three
    final taps on q), accumulated IN PLACE onto the open PSUM group
    (dense k=0 starts it, the last tap stops it), with PSUM->SBUF stage
    copies between waves (channel shift = free-dim offset + circular
    wrap cols; time shift = the T_d 0/1 stationary matrix, which also
    zeroes the causal boundary rows).
  - Software pipeline: stage waves of tile i-1 are emitted between dense
    chunks of tile i (q wave at chunk-step 0, final-tap wave at step 2,
    activations at step 3), giving cross-engine copies a full
    dense-chunk of slack.
  - When b_dense is all zeros (always true for this problem's inputs)
    the bias add is skipped; a general-bias variant compiles on demand.
  - y lands back in the h tile, a=tanh in the p1 tile, z in its own;
    y/a/z each leave in ONE [My, 2048] DMA per tile.
"""

import sys

if "/opt/trn_rl_repo" not in sys.path:
    sys.path.insert(0, "/opt/trn_rl_repo")

import numpy as np

B, L, DIN, F = 8, 2048, 1024, 2048
WC = 8            # conv taps
PAD = WC - 1      # 7
TS = 128 - PAD    # 121 output rows per time tile
NT = (L + TS - 1) // TS   # 17 time tiles
NCH = 4           # channel chunks of 512
CW = 512          # chunk width
KD = DIN // 128   # 8 contraction tiles
NBLK = L // 128   # 16 x row-blocks
NCORES = 8

# consts tensor column layout (one [128, 640] f32 input)
#   [d*128:(d+1)*128)  T_(d+1) down-shift (T[k, m] = 1 iff k == m-(d+1)),
#                      d = 0..3
#   [512:640)          identity (PE transpose)
CONST_COLS = 640
SHIFTS = (1, 2, 3, 4)
CONV3 = True   # 3-stage (1+S)(1+S^2)(1+S^4) vs 2-stage (1+S+S^2+S^3)(1+S^4)

_CACHE = {}


def _build_consts():
    c = np.zeros((128, CONST_COLS), np.float32)
    for si, d in enumerate(SHIFTS):
        for m in range(128):
            if m - d >= 0:
                c[m - d, si * 128 + m] = 1.0
    c[:, 512:640] = np.eye(128, dtype=np.float32)
    return c


def _split_matmul_waits(nc):
    """This walrus build accepts only one sync-wait command per instruction;
    hoist extra waits onto preceding same-engine no-ops (one wait each)."""
    import concourse.mybir as mybir

    for fn in nc.m.functions:
        for blk in fn.blocks:
            newl = []
            for inst in blk.instructions:
                si = getattr(inst, "sync_info", None)
                if (
                    si is not None
                    and len(si.on_wait) > 1
                    and not isinstance(inst, mybir.InstNoOp)
                    and getattr(inst, "engine", None) is not None
                ):
                    waits = list(si.on_wait)
                    for wi, w in enumerate(waits[:-1]):
                        pre = mybir.InstNoOp(
                            name=f"{inst.name}_wsplit{wi}",
                            sync_info=mybir.SyncInfo(on_wait=[w], on_update=[]),
                            bass_nofuse=True,
                            engine=inst.engine,
                        )
                        newl.append(pre)
                    si.on_wait = waits[-1:]
                newl.append(inst)
            blk.instructions = newl


def _build_nc(with_bias=False, mm_dtype_name="float32r", split_waits=True):
    import concourse.bass as bass
    import concourse.mybir as mybir
    from concourse import tile

    f32 = mybir.dt.float32
    mmdt = getattr(mybir.dt, mm_dtype_name)
    ADD = mybir.AluOpType.add
    ISGT = mybir.AluOpType.is_gt

    nc = bass.Bass("TRN2", target_bir_lowering=False, debug=False)

    x_d = nc.declare_dram_parameter("x", [L, DIN], f32, isOutput=False)
    w_d = nc.declare_dram_parameter("w", [DIN, F], f32, isOutput=False)
    bb_d = nc.declare_dram_parameter("bb", [128, F], f32, isOutput=False)
    cst_d = nc.declare_dram_parameter("cst", [128, CONST_COLS], f32, isOutput=False)
    y_d = nc.declare_dram_parameter("y", [L, F], f32, isOutput=True)
    a_d = nc.declare_dram_parameter("a", [L, F], f32, isOutput=True)
    z_d = nc.declare_dram_parameter("z", [L, F], f32, isOutput=True)

    def geom(i):
        t0 = TS * i
        if i == 0:
            hlo, row0 = 0, 0
        else:
            hlo, row0 = t0 - PAD, PAD
        Mh = min(L, t0 + TS) - hlo
        My = min(TS, L - t0)
        return t0, hlo, Mh, row0, My

    with tile.TileContext(nc) as tc:
        with (
            tc.tile_pool(name="cpool", bufs=1) as cpool,
            tc.tile_pool(name="wpool", bufs=1) as wpool,
            tc.tile_pool(name="xtpool", bufs=1) as xtpool,
            tc.tile_pool(name="xstage", bufs=2) as xstage,
            tc.tile_pool(name="hpool", bufs=3) as hpool,
            tc.tile_pool(name="p1pool", bufs=3) as p1pool,
            tc.tile_pool(name="zpool", bufs=2) as zpool,
            tc.tile_pool(name="psum", bufs=8, space="PSUM") as psum,
        ):
            cst = cpool.tile([128, CONST_COLS], mmdt, tag="cst", name="cst")
            nc.sync.dma_start(cst[:], cst_d[:].bitcast(mmdt))
            bb = None
            if with_bias:
                bb = cpool.tile([128, F], f32, tag="bb", name="bb")
                nc.sync.dma_start(bb[:], bb_d[:])

            ident = cst[:, 512:640].bitcast(f32)

            def tshift(d, Mh):
                si = SHIFTS.index(d)
                return cst[0:Mh, si * 128 : si * 128 + Mh]

            xt = [
                xtpool.tile([128, L], mmdt, tag=f"xt{k}", name=f"xt{k}")
                for k in range(KD)
            ]

            xs_tiles = {}
            fetched = set()
            transposed = set()

            def fetch_block(j):
                if j in fetched or j >= NBLK:
                    return
                fetched.add(j)
                xs = xstage.tile([128, DIN], f32, tag="xs", name=f"xs{j}")
                nc.sync.dma_start(xs[:], x_d[j * 128 : (j + 1) * 128, :])
                xs_tiles[j] = xs

            # W as 32 [128, CW] tiles, chunk-major DMA order so chunk-0
            # columns (all k) arrive first and the first dense can start.
            # Each chunk's 8 DMAs are dispatched from a different engine:
            # the engine-side descriptor write costs ~1.3us per DMA, and
            # serializing all 32 on Sync would starve the x-block fetches.
            fetch_block(0)
            fetch_block(1)
            wt = [[None] * NCH for _ in range(KD)]
            w_engines = [nc.sync, nc.scalar, nc.gpsimd, nc.gpsimd]
            for n in range(NCH):
                for k in range(KD):
                    t = wpool.tile([128, CW], mmdt, tag=f"w{k}_{n}", name=f"w{k}_{n}")
                    w_engines[n].dma_start(
                        t[:],
                        w_d[k * 128 : (k + 1) * 128, n * CW : (n + 1) * CW].bitcast(mmdt),
                    )
                    wt[k][n] = t

            def transpose_block(j, ps):
                # 8 PE transposes of x block j into ps[0], ps[1] (which the
                # dense matmuls will overwrite afterwards), then copies to xt
                xs = xs_tiles.pop(j)
                transposed.add(j)
                for half in range(2):
                    tp = ps[half]
                    for q in range(4):
                        k = half * 4 + q
                        nc.tensor.transpose(
                            tp[:, q * 128 : (q + 1) * 128],
                            xs[:, k * 128 : (k + 1) * 128],
                            ident,
                        )
                    for q in range(4):
                        k = half * 4 + q
                        nc.vector.tensor_copy(
                            xt[k][:, j * 128 : (j + 1) * 128],
                            tp[:, q * 128 : (q + 1) * 128],
                        )

            def need_blocks(i):
                if not (0 <= i < NT):
                    return []
                _, hlo, Mh, _, _ = geom(i)
                return list(range(hlo // 128, (hlo + Mh - 1) // 128 + 1))

            live = {}

            def open_iter(i):
                live[i] = {
                    "g": geom(i),
                    "ps": [psum.tile([128, CW], f32, tag="hp", name=f"hp{i}_{c}") for c in range(NCH)],
                    "hs": hpool.tile([128, F + PAD], mmdt, tag="hs", name=f"hs{i}"),
                    "p1s": p1pool.tile([128, F + PAD], mmdt, tag="p1s", name=f"p1s{i}"),
                    "zs": zpool.tile([128, F], f32, tag="zs", name=f"zs{i}"),
                }

            def emit_dense_chunk(i, n):
                st = live[i]
                _, hlo, Mh, _, _ = st["g"]
                hp = st["ps"][n]
                for k in range(KD):
                    nc.tensor.matmul(
                        hp[0:Mh, :],
                        xt[k][:, hlo : hlo + Mh],
                        wt[k][n][:],
                        start=(k == 0),
                        stop=False,
                    )

            def emit_hadd(i, n):
                st = live[i]
                Mh = st["g"][2]
                hs = st["hs"]
                if with_bias:
                    nc.vector.tensor_tensor(
                        hs[0:Mh, n * CW : (n + 1) * CW],
                        st["ps"][n][0:Mh, :],
                        bb[0:Mh, n * CW : (n + 1) * CW],
                        ADD,
                    )
                else:
                    nc.vector.tensor_copy(
                        hs[0:Mh, n * CW : (n + 1) * CW], st["ps"][n][0:Mh, :]
                    )
                if n == 0:
                    nc.vector.tensor_copy(hs[0:Mh, F : F + PAD], hs[0:Mh, 0:PAD])

            def emit_stage_mm(i, n, d, src, stop=False):
                # accumulate S^d(src) onto chunk n's open PSUM group
                st = live[i]
                Mh = st["g"][2]
                nc.tensor.matmul(
                    st["ps"][n][0:Mh, :],
                    tshift(d, Mh),
                    src[0:Mh, n * CW + d : n * CW + d + CW],
                    start=False,
                    stop=stop,
                )

            def emit_stage_copy(i, n, dst, wrapw):
                # psum -> sbuf stage copy, split across both engines: the
                # next wave is gated on it, so latency matters
                st = live[i]
                Mh = st["g"][2]
                half = CW // 2
                nc.scalar.copy(
                    dst[0:Mh, n * CW : n * CW + half],
                    st["ps"][n][0:Mh, 0:half],
                )
                nc.vector.tensor_copy(
                    dst[0:Mh, n * CW + half : (n + 1) * CW],
                    st["ps"][n][0:Mh, half:CW],
                )
                if n == 0:
                    nc.vector.tensor_copy(
                        dst[0:Mh, F : F + wrapw], dst[0:Mh, 0:wrapw]
                    )

            def emit_qwave(i, n):
                st = live[i]
                if CONV3:
                    # p1 = h + S h
                    emit_stage_mm(i, n, 1, st["hs"])
                    emit_stage_copy(i, n, st["p1s"], 6)
                else:
                    # q = h + S^4 h
                    emit_stage_mm(i, n, 4, st["hs"])
                    emit_stage_copy(i, n, st["p1s"], 3)

            def emit_midwave(i, n):
                # CONV3 only: p2 = p1 + S^2 p1, written back into hs
                st = live[i]
                emit_stage_mm(i, n, 2, st["p1s"])
                emit_stage_copy(i, n, st["hs"], 4)

            def emit_finalwave(i, n):
                st = live[i]
                if CONV3:
                    # y = p2 + S^4 p2 (p2 lives in hs)
                    emit_stage_mm(i, n, 4, st["hs"], stop=True)
                else:
                    # y = q + S q + S^2 q + S^3 q
                    for d in (1, 2, 3):
                        emit_stage_mm(i, n, d, st["p1s"], stop=(d == 3))

            def emit_outputs_chunk(i, n):
                st = live[i]
                t0, hlo, Mh, row0, My = st["g"]
                hp = st["ps"][n]
                cols = slice(n * CW, (n + 1) * CW)
                # full-row copies (engine APs must start at partition 0);
                # the DMA slices the valid [row0, row0+My) window.
                # y back into hs (dead after the final wave), a into p1s
                if n < 2:
                    nc.scalar.copy(st["hs"][0:Mh, cols], hp[0:Mh, :])
                else:
                    nc.vector.tensor_copy(st["hs"][0:Mh, cols], hp[0:Mh, :])
                nc.scalar.activation(
                    st["p1s"][0:Mh, cols],
                    hp[0:Mh, :],
                    mybir.ActivationFunctionType.Tanh,
                )
                nc.vector.tensor_scalar(
                    st["zs"][0:Mh, cols],
                    st["hs"][0:Mh, cols].bitcast(f32),
                    0.0,
                    None,
                    ISGT,
                )

            def emit_dma_out(i, cs):
                st = live[i]
                t0, hlo, Mh, row0, My = st["g"]
                rows = slice(row0, row0 + My)
                nc.gpsimd.dma_start(
                    y_d[t0 : t0 + My, cs], st["hs"][rows, cs].bitcast(f32)
                )
                nc.gpsimd.dma_start(
                    a_d[t0 : t0 + My, cs], st["p1s"][rows, cs].bitcast(f32)
                )
                nc.gpsimd.dma_start(z_d[t0 : t0 + My, cs], st["zs"][rows, cs])

            def emit_dma_out_half(i, half):
                emit_dma_out(i, slice(half * (F // 2), (half + 1) * (F // 2)))

            for i in range(NT + 1):
                cur = i if i < NT else None
                prv = i - 1 if i >= 1 else None
                if cur is not None:
                    open_iter(cur)
                    for j in need_blocks(cur):
                        if j not in transposed:
                            transpose_block(j, live[cur]["ps"][0:2])
                    for j in (
                        need_blocks(cur + 1)
                        + need_blocks(cur + 2)
                        + need_blocks(cur + 3)
                    ):
                        fetch_block(j)  # prefetch x blocks three iters ahead
                last = prv == NT - 1
                for step in range(NCH):
                    if cur is not None:
                        emit_dense_chunk(cur, step)
                        emit_hadd(cur, step)
                    if prv is not None:
                        if step == 0:
                            for c in range(NCH):
                                emit_qwave(prv, c)
                        elif step == 2:
                            if CONV3:
                                for c in range(NCH):
                                    emit_midwave(prv, c)
                            else:
                                for c in range(NCH):
                                    emit_finalwave(prv, c)
                                for c in (0, 1):
                                    emit_outputs_chunk(prv, c)
                                    if last:
                                        emit_dma_out(prv, slice(c * CW, (c + 1) * CW))
                        elif step == 3:
                            if CONV3:
                                for c in range(NCH):
                                    emit_finalwave(prv, c)
                                for c in range(NCH):
                                    emit_outputs_chunk(prv, c)
                                    if last:
                                        emit_dma_out(prv, slice(c * CW, (c + 1) * CW))
                            else:
                                if not last:
                                    emit_dma_out_half(prv, 0)
                                for c in (2, 3):
                                    emit_outputs_chunk(prv, c)
                                    if last:
                                        emit_dma_out(prv, slice(c * CW, (c + 1) * CW))
                if prv is not None:
                    if not last:
                        if CONV3:
                            emit_dma_out_half(prv, 0)
                        emit_dma_out_half(prv, 1)
                    live.pop(prv)

    if split_waits:
        _split_matmul_waits(nc)
    return nc


def _get_nc(with_bias=False):
    key = ("nc", with_bias)
    if key not in _CACHE:
        _CACHE[key] = _build_nc(with_bias=with_bias)
    return _CACHE[key]


def _make_in_maps(x, W, b):
    x = np.asarray(x, np.float32)
    W = np.ascontiguousarray(np.asarray(W, np.float32))
    b = np.asarray(b, np.float32)
    bb = np.ascontiguousarray(np.broadcast_to(b, (128, F)))
    cst = _build_consts()
    return [
        {"x": np.ascontiguousarray(x[n]), "w": W, "bb": bb, "cst": cst}
        for n in range(NCORES)
    ]


def kernel(x, W_dense, b_dense):
    from concourse.bass_utils import run_bass_kernel_spmd

    nc = _get_nc(with_bias=bool(np.any(np.asarray(b_dense))))
    in_maps = _make_in_maps(x, W_dense, b_dense)
    res = run_bass_kernel_spmd(nc, in_maps, list(range(NCORES))).results

    y = np.stack([res[n]["y"] for n in range(NCORES)])
    a = np.stack([res[n]["a"] for n in range(NCORES)])
    z = np.stack([res[n]["z"] for n in range(NCORES)])
    return y, a, z


# revision 18
# speedup vs baseline: 1.1353x; 1.1353x over previous
"""Trainium2 Bass kernel for nn_ConvShiftLayer.

Computes, per batch element n:
    h = x[n] @ W_dense + b_dense                      (2048, 2048)
    y[t, o] = sum_{d=0..7} h[t-d, (o+d) % 2048]       (h[<0] = 0)
    a = tanh(y),  z = (y > 0) as f32
Returns (y, a, z) each of shape (8, 2048, 2048) f32.

Strategy: data-parallel over batch, 1 element per NeuronCore (8 cores).
Per core:
  - x row-blocks are PE-transposed to xT (D on partitions), one block per
    tile iteration, written into that iteration's chunk PSUM tiles before
    the dense matmuls claim them - so TensorE never waits on the full x DMA.
  - h = xT.T @ W via fp32r matmuls (full-rate) into PSUM; kept there.
  - The 8-tap shifted sum uses the factorization
        y = (1 + S + S^2 + S^3)(1 + S^4) h,   S q[t,o] = q[t-1, o+1]
    as 4 shift-matrix matmuls per 512-chunk (q = h + S^4 h, then three
    final taps on q), accumulated IN PLACE onto the open PSUM group
    (dense k=0 starts it, the last tap stops it), with PSUM->SBUF stage
    copies between waves (channel shift = free-dim offset + circular
    wrap cols; time shift = the T_d 0/1 stationary matrix, which also
    zeroes the causal boundary rows).
  - Software pipeline: stage waves of tile i-1 are emitted between dense
    chunks of tile i (q wave at chunk-step 0, final-tap wave at step 2,
    activations at step 3), giving cross-engine copies a full
    dense-chunk of slack.
  - When b_dense is all zeros (always true for this problem's inputs)
    the bias add is skipped; a general-bias variant compiles on demand.
  - y lands back in the h tile, a=tanh in the p1 tile, z in its own;
    y/a/z each leave in ONE [My, 2048] DMA per tile.
"""

import sys

if "/opt/trn_rl_repo" not in sys.path:
    sys.path.insert(0, "/opt/trn_rl_repo")

import numpy as np

B, L, DIN, F = 8, 2048, 1024, 2048
WC = 8            # conv taps
PAD = WC - 1      # 7
TS = 128 - PAD    # 121 output rows per time tile
NT = (L + TS - 1) // TS   # 17 time tiles
NCH = 4           # channel chunks of 512
CW = 512          # chunk width
KD = DIN // 128   # 8 contraction tiles
NBLK = L // 128   # 16 x row-blocks
NCORES = 8

# consts tensor column layout (one [128, 640] f32 input)
#   [d*128:(d+1)*128)  T_(d+1) down-shift (T[k, m] = 1 iff k == m-(d+1)),
#                      d = 0..3
#   [512:640)          identity (PE transpose)
CONST_COLS = 640
SHIFTS = (1, 2, 3, 4)
CONV3 = False  # 3-stage (1+S)(1+S^2)(1+S^4) vs 2-stage (1+S+S^2+S^3)(1+S^4)

_CACHE = {}


def _build_consts():
    c = np.zeros((128, CONST_COLS), np.float32)
    for si, d in enumerate(SHIFTS):
        for m in range(128):
            if m - d >= 0:
                c[m - d, si * 128 + m] = 1.0
    c[:, 512:640] = np.eye(128, dtype=np.float32)
    return c


def _split_matmul_waits(nc):
    """This walrus build accepts only one sync-wait command per instruction;
    hoist extra waits onto preceding same-engine no-ops (one wait each)."""
    import concourse.mybir as mybir

    for fn in nc.m.functions:
        for blk in fn.blocks:
            newl = []
            for inst in blk.instructions:
                si = getattr(inst, "sync_info", None)
                if (
                    si is not None
                    and len(si.on_wait) > 1
                    and not isinstance(inst, mybir.InstNoOp)
                    and getattr(inst, "engine", None) is not None
                ):
                    waits = list(si.on_wait)
                    for wi, w in enumerate(waits[:-1]):
                        pre = mybir.InstNoOp(
                            name=f"{inst.name}_wsplit{wi}",
                            sync_info=mybir.SyncInfo(on_wait=[w], on_update=[]),
                            bass_nofuse=True,
                            engine=inst.engine,
                        )
                        newl.append(pre)
                    si.on_wait = waits[-1:]
                newl.append(inst)
            blk.instructions = newl


def _build_nc(with_bias=False, mm_dtype_name="float32r", split_waits=True):
    import concourse.bass as bass
    import concourse.mybir as mybir
    from concourse import tile

    f32 = mybir.dt.float32
    mmdt = getattr(mybir.dt, mm_dtype_name)
    ADD = mybir.AluOpType.add
    ISGT = mybir.AluOpType.is_gt

    nc = bass.Bass("TRN2", target_bir_lowering=False, debug=False)

    x_d = nc.declare_dram_parameter("x", [L, DIN], f32, isOutput=False)
    w_d = nc.declare_dram_parameter("w", [DIN, F], f32, isOutput=False)
    bb_d = nc.declare_dram_parameter("bb", [128, F], f32, isOutput=False)
    cst_d = nc.declare_dram_parameter("cst", [128, CONST_COLS], f32, isOutput=False)
    y_d = nc.declare_dram_parameter("y", [L, F], f32, isOutput=True)
    a_d = nc.declare_dram_parameter("a", [L, F], f32, isOutput=True)
    z_d = nc.declare_dram_parameter("z", [L, F], f32, isOutput=True)

    def geom(i):
        t0 = TS * i
        if i == 0:
            hlo, row0 = 0, 0
        else:
            hlo, row0 = t0 - PAD, PAD
        Mh = min(L, t0 + TS) - hlo
        My = min(TS, L - t0)
        return t0, hlo, Mh, row0, My

    with tile.TileContext(nc) as tc:
        with (
            tc.tile_pool(name="cpool", bufs=1) as cpool,
            tc.tile_pool(name="wpool", bufs=1) as wpool,
            tc.tile_pool(name="xtpool", bufs=1) as xtpool,
            tc.tile_pool(name="xstage", bufs=2) as xstage,
            tc.tile_pool(name="hpool", bufs=3) as hpool,
            tc.tile_pool(name="p1pool", bufs=3) as p1pool,
            tc.tile_pool(name="zpool", bufs=2) as zpool,
            tc.tile_pool(name="psum", bufs=8, space="PSUM") as psum,
        ):
            cst = cpool.tile([128, CONST_COLS], mmdt, tag="cst", name="cst")
            nc.sync.dma_start(cst[:], cst_d[:].bitcast(mmdt))
            bb = None
            if with_bias:
                bb = cpool.tile([128, F], f32, tag="bb", name="bb")
                nc.sync.dma_start(bb[:], bb_d[:])

            ident = cst[:, 512:640].bitcast(f32)

            def tshift(d, Mh):
                si = SHIFTS.index(d)
                return cst[0:Mh, si * 128 : si * 128 + Mh]

            xt = [
                xtpool.tile([128, L], mmdt, tag=f"xt{k}", name=f"xt{k}")
                for k in range(KD)
            ]

            xs_tiles = {}
            fetched = set()
            transposed = set()

            def fetch_block(j):
                if j in fetched or j >= NBLK:
                    return
                fetched.add(j)
                xs = xstage.tile([128, DIN], f32, tag="xs", name=f"xs{j}")
                nc.sync.dma_start(xs[:], x_d[j * 128 : (j + 1) * 128, :])
                xs_tiles[j] = xs

            # W as 32 [128, CW] tiles, chunk-major DMA order so chunk-0
            # columns (all k) arrive first and the first dense can start.
            # Each chunk's 8 DMAs are dispatched from a different engine:
            # the engine-side descriptor write costs ~1.3us per DMA, and
            # serializing all 32 on Sync would starve the x-block fetches.
            fetch_block(0)
            fetch_block(1)
            wt = [[None] * NCH for _ in range(KD)]
            w_engines = [nc.sync, nc.scalar, nc.gpsimd, nc.gpsimd]
            for n in range(NCH):
                for k in range(KD):
                    t = wpool.tile([128, CW], mmdt, tag=f"w{k}_{n}", name=f"w{k}_{n}")
                    w_engines[n].dma_start(
                        t[:],
                        w_d[k * 128 : (k + 1) * 128, n * CW : (n + 1) * CW].bitcast(mmdt),
                    )
                    wt[k][n] = t

            def transpose_block(j, ps):
                # 8 PE transposes of x block j into ps[0], ps[1] (which the
                # dense matmuls will overwrite afterwards), then copies to xt
                xs = xs_tiles.pop(j)
                transposed.add(j)
                for half in range(2):
                    tp = ps[half]
                    for q in range(4):
                        k = half * 4 + q
                        nc.tensor.transpose(
                            tp[:, q * 128 : (q + 1) * 128],
                            xs[:, k * 128 : (k + 1) * 128],
                            ident,
                        )
                    for q in range(4):
                        k = half * 4 + q
                        nc.vector.tensor_copy(
                            xt[k][:, j * 128 : (j + 1) * 128],
                            tp[:, q * 128 : (q + 1) * 128],
                        )

            def need_blocks(i):
                if not (0 <= i < NT):
                    return []
                _, hlo, Mh, _, _ = geom(i)
                return list(range(hlo // 128, (hlo + Mh - 1) // 128 + 1))

            live = {}

            def open_iter(i):
                live[i] = {
                    "g": geom(i),
                    "ps": [psum.tile([128, CW], f32, tag="hp", name=f"hp{i}_{c}") for c in range(NCH)],
                    "hs": hpool.tile([128, F + PAD], mmdt, tag="hs", name=f"hs{i}"),
                    "p1s": p1pool.tile([128, F + PAD], mmdt, tag="p1s", name=f"p1s{i}"),
                    "zs": zpool.tile([128, F], f32, tag="zs", name=f"zs{i}"),
                }

            def emit_dense_chunk(i, n):
                st = live[i]
                _, hlo, Mh, _, _ = st["g"]
                hp = st["ps"][n]
                for k in range(KD):
                    nc.tensor.matmul(
                        hp[0:Mh, :],
                        xt[k][:, hlo : hlo + Mh],
                        wt[k][n][:],
                        start=(k == 0),
                        stop=False,
                    )

            def emit_hadd(i, n):
                st = live[i]
                Mh = st["g"][2]
                hs = st["hs"]
                if with_bias:
                    nc.vector.tensor_tensor(
                        hs[0:Mh, n * CW : (n + 1) * CW],
                        st["ps"][n][0:Mh, :],
                        bb[0:Mh, n * CW : (n + 1) * CW],
                        ADD,
                    )
                else:
                    nc.vector.tensor_copy(
                        hs[0:Mh, n * CW : (n + 1) * CW], st["ps"][n][0:Mh, :]
                    )
                if n == 0:
                    nc.vector.tensor_copy(hs[0:Mh, F : F + PAD], hs[0:Mh, 0:PAD])

            def emit_stage_mm(i, n, d, src, stop=False):
                # accumulate S^d(src) onto chunk n's open PSUM group
                st = live[i]
                Mh = st["g"][2]
                nc.tensor.matmul(
                    st["ps"][n][0:Mh, :],
                    tshift(d, Mh),
                    src[0:Mh, n * CW + d : n * CW + d + CW],
                    start=False,
                    stop=stop,
                )

            def emit_stage_copy(i, n, dst, wrapw):
                # psum -> sbuf stage copy, split across both engines: the
                # next wave is gated on it, so latency matters
                st = live[i]
                Mh = st["g"][2]
                half = CW // 2
                nc.scalar.copy(
                    dst[0:Mh, n * CW : n * CW + half],
                    st["ps"][n][0:Mh, 0:half],
                )
                nc.vector.tensor_copy(
                    dst[0:Mh, n * CW + half : (n + 1) * CW],
                    st["ps"][n][0:Mh, half:CW],
                )
                if n == 0:
                    nc.vector.tensor_copy(
                        dst[0:Mh, F : F + wrapw], dst[0:Mh, 0:wrapw]
                    )

            def emit_qwave(i, n):
                st = live[i]
                if CONV3:
                    # p1 = h + S h
                    emit_stage_mm(i, n, 1, st["hs"])
                    emit_stage_copy(i, n, st["p1s"], 6)
                else:
                    # q = h + S^4 h
                    emit_stage_mm(i, n, 4, st["hs"])
                    emit_stage_copy(i, n, st["p1s"], 3)

            def emit_midwave(i, n):
                # CONV3 only: p2 = p1 + S^2 p1, written back into hs
                st = live[i]
                emit_stage_mm(i, n, 2, st["p1s"])
                emit_stage_copy(i, n, st["hs"], 4)

            def emit_finalwave(i, n):
                st = live[i]
                if CONV3:
                    # y = p2 + S^4 p2 (p2 lives in hs)
                    emit_stage_mm(i, n, 4, st["hs"], stop=True)
                else:
                    # y = q + S q + S^2 q + S^3 q
                    for d in (1, 2, 3):
                        emit_stage_mm(i, n, d, st["p1s"], stop=(d == 3))

            def emit_outputs_chunk(i, n):
                st = live[i]
                t0, hlo, Mh, row0, My = st["g"]
                hp = st["ps"][n]
                cols = slice(n * CW, (n + 1) * CW)
                # full-row copies (engine APs must start at partition 0);
                # the DMA slices the valid [row0, row0+My) window.
                # y back into hs (dead after the final wave), a into p1s
                if n < 2:
                    nc.scalar.copy(st["hs"][0:Mh, cols], hp[0:Mh, :])
                else:
                    nc.vector.tensor_copy(st["hs"][0:Mh, cols], hp[0:Mh, :])
                nc.scalar.activation(
                    st["p1s"][0:Mh, cols],
                    hp[0:Mh, :],
                    mybir.ActivationFunctionType.Tanh,
                )
                nc.vector.tensor_scalar(
                    st["zs"][0:Mh, cols],
                    st["hs"][0:Mh, cols].bitcast(f32),
                    0.0,
                    None,
                    ISGT,
                )

            def emit_dma_out(i, cs):
                st = live[i]
                t0, hlo, Mh, row0, My = st["g"]
                rows = slice(row0, row0 + My)
                nc.gpsimd.dma_start(
                    y_d[t0 : t0 + My, cs], st["hs"][rows, cs].bitcast(f32)
                )
                nc.gpsimd.dma_start(
                    a_d[t0 : t0 + My, cs], st["p1s"][rows, cs].bitcast(f32)
                )
                nc.gpsimd.dma_start(z_d[t0 : t0 + My, cs], st["zs"][rows, cs])

            def emit_dma_out_half(i, half):
                emit_dma_out(i, slice(half * (F // 2), (half + 1) * (F // 2)))

            # PE warm-up: the HAM clock gate starts at 1.2 GHz and only
            # reaches 2.4 GHz after ~3.4us of sustained activity; the input
            # DMAs take ~10us, so burn that window with tiny matmuls on the
            # already-resident consts to enter the loop at full clock.
            warm = psum.tile([128, CW], f32, tag="hp", name="warm")
            for _ in range(40):
                nc.tensor.matmul(
                    warm[0:128, 0:64],
                    cst[0:128, 0:128],
                    cst[0:128, 0:64],
                    start=True,
                    stop=True,
                )

            for i in range(NT + 1):
                cur = i if i < NT else None
                prv = i - 1 if i >= 1 else None
                if cur is not None:
                    open_iter(cur)
                    for j in need_blocks(cur):
                        if j not in transposed:
                            transpose_block(j, live[cur]["ps"][0:2])
                    for j in (
                        need_blocks(cur + 1)
                        + need_blocks(cur + 2)
                        + need_blocks(cur + 3)
                    ):
                        fetch_block(j)  # prefetch x blocks three iters ahead
                last = prv == NT - 1
                for step in range(NCH):
                    if cur is not None:
                        emit_dense_chunk(cur, step)
                        emit_hadd(cur, step)
                    if prv is not None:
                        if step == 0:
                            for c in range(NCH):
                                emit_qwave(prv, c)
                        elif step == 2:
                            if CONV3:
                                for c in range(NCH):
                                    emit_midwave(prv, c)
                            else:
                                for c in range(NCH):
                                    emit_finalwave(prv, c)
                                for c in (0, 1):
                                    emit_outputs_chunk(prv, c)
                                    if last:
                                        emit_dma_out(prv, slice(c * CW, (c + 1) * CW))
                        elif step == 3:
                            if CONV3:
                                for c in range(NCH):
                                    emit_finalwave(prv, c)
                                for c in range(NCH):
                                    emit_outputs_chunk(prv, c)
                                    if last:
                                        emit_dma_out(prv, slice(c * CW, (c + 1) * CW))
                            else:
                                if not last:
                                    emit_dma_out_half(prv, 0)
                                for c in (2, 3):
                                    emit_outputs_chunk(prv, c)
                                    if last:
                                        emit_dma_out(prv, slice(c * CW, (c + 1) * CW))
                if prv is not None:
                    if not last:
                        if CONV3:
                            emit_dma_out_half(prv, 0)
                        emit_dma_out_half(prv, 1)
                    live.pop(prv)

    if split_waits:
        _split_matmul_waits(nc)
    return nc


def _get_nc(with_bias=False):
    key = ("nc", with_bias)
    if key not in _CACHE:
        _CACHE[key] = _build_nc(with_bias=with_bias)
    return _CACHE[key]


def _make_in_maps(x, W, b):
    x = np.asarray(x, np.float32)
    W = np.ascontiguousarray(np.asarray(W, np.float32))
    b = np.asarray(b, np.float32)
    bb = np.ascontiguousarray(np.broadcast_to(b, (128, F)))
    cst = _build_consts()
    return [
        {"x": np.ascontiguousarray(x[n]), "w": W, "bb": bb, "cst": cst}
        for n in range(NCORES)
    ]


def kernel(x, W_dense, b_dense):
    from concourse.bass_utils import run_bass_kernel_spmd

    nc = _get_nc(with_bias=bool(np.any(np.asarray(b_dense))))
    in_maps = _make_in_maps(x, W_dense, b_dense)
    res = run_bass_kernel_spmd(nc, in_maps, list(range(NCORES))).results

    y = np.stack([res[n]["y"] for n in range(NCORES)])
    a = np.stack([res[n]["a"] for n in range(NCORES)])
    z = np.stack([res[n]["z"] for n in range(NCORES)])
    return y, a, z
